# Initial kernel scaffold
#
"""DendriticBranchLayerSparse kernel for TRN2 (8 NeuronCores, batch-sharded).

out[b, o] = sum_{k<4} x[b, 4o+k] * w[4o+k]  +  t[b] * tw[o]

Layout: batch on partitions (128 rows/core). Per chunk of 4096 x-columns:
  PE broadcasts w across partitions (K=1 matmul vs ones) and computes the
  rank-1 bias t (x) tw into PSUM; ACT moves broadcast w PSUM->SBUF; DVE does
  the elementwise multiply and the final bias add; GPSIMD does the two
  pair-sum levels of the 4->1 segment reduce.
"""

import sys

if "/opt/trn_rl_repo" not in sys.path:
    sys.path.insert(0, "/opt/trn_rl_repo")

import numpy as np

B, NIN, NOUT, BF = 1024, 32768, 8192, 4
NC = 8
BS = B // NC  # 128 batch rows per core == SBUF partitions
CHUNK = 4096  # x columns per pipeline chunk
OCHUNK = CHUNK // BF  # 1024 outputs per chunk
NCHUNKS = NIN // CHUNK

_cache = {}


def _build():
    import concourse.bass as bass
    import concourse.mybir as mybir
    from concourse.tile import TileContext

    f32 = mybir.dt.float32
    nc = bass.Bass()
    x = nc.declare_dram_parameter("x", [BS, NIN], f32, isOutput=False)
    t = nc.declare_dram_parameter("t", [BS], f32, isOutput=False)
    w = nc.declare_dram_parameter("w", [NIN], f32, isOutput=False)
    tw = nc.declare_dram_parameter("tw", [NOUT], f32, isOutput=False)
    out = nc.declare_dram_parameter("out", [BS, NOUT], f32, isOutput=True)

    with TileContext(nc) as tc:
        with (
            tc.tile_pool(name="const", bufs=1) as cpool,
            tc.tile_pool(name="stream", bufs=2) as spool,
            tc.tile_pool(name="mid", bufs=2) as mpool,
            tc.tile_pool(name="psw", bufs=2, space="PSUM") as pw_pool,
            tc.tile_pool(name="psb", bufs=2, space="PSUM") as pb_pool,
        ):
            ones = cpool.tile([1, BS], f32)
            nc.vector.memset(ones[:], 1.0)
            t_row = cpool.tile([1, BS], f32)
            nc.sync.dma_start(out=t_row[:], in_=t[:].rearrange("(a b) -> a b", a=1))

            for c in range(NCHUNKS):
                x_c = spool.tile([BS, CHUNK], f32, tag="x")
                nc.sync.dma_start(out=x_c[:], in_=x[:, c * CHUNK : (c + 1) * CHUNK])
                w_row = spool.tile([1, CHUNK], f32, tag="wrow")
                nc.sync.dma_start(
                    out=w_row[:],
                    in_=w[c * CHUNK : (c + 1) * CHUNK].rearrange("(a b) -> a b", a=1),
                )
                tw_row = spool.tile([1, OCHUNK], f32, tag="twrow")
                nc.sync.dma_start(
                    out=tw_row[:],
                    in_=tw[c * OCHUNK : (c + 1) * OCHUNK].rearrange(
                        "(a b) -> a b", a=1
                    ),
                )

                # broadcast w chunk to all 128 partitions: PE (ones^T @ w) -> PSUM,
                # then ACT copies PSUM -> SBUF 512 cols at a time.
                w_bc = spool.tile([BS, CHUNK], f32, tag="wbc")
                for j in range(CHUNK // 512):
                    ps_w = pw_pool.tile([BS, 512], f32, tag="psw")
                    nc.tensor.matmul(
                        ps_w[:],
                        ones[:],
                        w_row[:, j * 512 : (j + 1) * 512],
                        start=True,
                        stop=True,
                    )
                    nc.scalar.copy(out=w_bc[:, j * 512 : (j + 1) * 512], in_=ps_w[:])

                y = spool.tile([BS, CHUNK], f32, tag="y")
                nc.vector.tensor_mul(out=y[:], in0=x_c[:], in1=w_bc[:])

                s = mpool.tile([BS, CHUNK // 2], f32, tag="s")
                nc.gpsimd.tensor_add(out=s[:], in0=y[:, 0::2], in1=y[:, 1::2])
                u = mpool.tile([BS, OCHUNK], f32, tag="u")
                nc.gpsimd.tensor_add(out=u[:], in0=s[:, 0::2], in1=s[:, 1::2])

                # rank-1 bias t (x) tw via PE into PSUM
                ps_b = pb_pool.tile([BS, OCHUNK], f32, tag="psb")
                for j in range(OCHUNK // 512):
                    nc.tensor.matmul(
                        ps_b[:, j * 512 : (j + 1) * 512],
                        t_row[:],
                        tw_row[:, j * 512 : (j + 1) * 512],
                        start=True,
                        stop=True,
                    )

                outt = mpool.tile([BS, OCHUNK], f32, tag="outt")
                nc.vector.tensor_add(out=outt[:], in0=u[:], in1=ps_b[:])
                nc.sync.dma_start(
                    out=out[:, c * OCHUNK : (c + 1) * OCHUNK], in_=outt[:]
                )
    return nc


def get_nc():
    if "nc" not in _cache:
        _cache["nc"] = _build()
    return _cache["nc"]


def kernel(x, t, weight_vals, t_weights):
    from concourse.bass_utils import run_bass_kernel_spmd

    nc = get_nc()
    x = np.ascontiguousarray(np.asarray(x, dtype=np.float32))
    t = np.ascontiguousarray(np.asarray(t, dtype=np.float32))
    w = np.ascontiguousarray(np.asarray(weight_vals, dtype=np.float32))
    tw = np.ascontiguousarray(np.asarray(t_weights, dtype=np.float32).reshape(NOUT))
    in_maps = [
        {
            "x": x[i * BS : (i + 1) * BS],
            "t": t[i * BS : (i + 1) * BS],
            "w": w,
            "tw": tw,
        }
        for i in range(NC)
    ]
    res = run_bass_kernel_spmd(nc, in_maps, list(range(NC)))
    return np.concatenate([r["out"] for r in res.results], axis=0)


# revision 26
# speedup vs baseline: 1.0253x; 1.0253x over previous
"""DendriticBranchLayerSparse kernel for TRN2 (8 NeuronCores, batch-sharded).

out[b, o] = sum_{k<4} x[b, 4o+k] * w[4o+k]  +  t[b] * tw[o]

Layout (v6, fp16 datapath): host packs each core's x shard as
xti [128, 256*128] fp16 where xti[p, g*128 + b] = x[b, g*128 + p] --
feature-on-partition, 128-feature blocks g along the free dim. All
device DMAs are fully contiguous; fp16 halves the dominant x stream.

Per 4096-column chunk (32 feature blocks):
  - DVE: ONE tensor_tensor multiply per chunk, in-place:
    y[p, g, b] = x[p, g, b] * w[p, g] via a step-0 broadcast AP on w.
  - PE: per 128-output group (32-partition sub-range m of a PSUM bank,
    tile_position=(0, 32m)): a K=1 bias matmul (lhsT = tw slice [1, 32],
    rhs = t row [1, 128]) opens the accumulation group with tw[o]*t[b];
    a K=128 reduce matmul (lhsT = 0/1 block-diagonal [128, 32])
    accumulates the segment sums in fp32 PSUM and closes it.
    4 groups stack on partitions {0,32,64,96}; 4 output groups along the
    free dim share one [128, 512] PSUM bank.
  - ACT copies each full bank PSUM->SBUF casting to fp16 (FD=512 on all
    128 partitions); the idle GPSIMD (SWDGE) DMAs it out contiguously.
  - Host casts back to fp32 and un-permutes.

A post-pass moves excess semaphore waits onto NoOps (walrus fits only one
wait on several instruction structs).
"""

import sys

if "/opt/trn_rl_repo" not in sys.path:
    sys.path.insert(0, "/opt/trn_rl_repo")

import numpy as np

B, NIN, NOUT, BF = 1024, 32768, 8192, 4
NC = 8
BS = B // NC  # 128 batch rows per core
FBLK = 128  # features per block (partition dim)
NBLK = NIN // FBLK  # 256 feature blocks
SUPER = 4096  # features per input DMA chunk (1 MiB in fp16)
BLKS_PER_SUPER = SUPER // FBLK  # 32
NSUPER = NIN // SUPER  # 8
NGQ = NBLK // 4  # 64 128-output groups
NBANK = NGQ // 4  # 16 PSUM bank tiles (512 outputs each)

_cache = {}


def _build(reps=1):
    import concourse.bass as bass
    import concourse.mybir as mybir
    from concourse.tile import TileContext

    f16 = mybir.dt.float16
    f32 = mybir.dt.float32
    nc = bass.Bass()
    xti = nc.declare_dram_parameter("xti", [FBLK, NBLK * BS], f16, isOutput=False)
    wmat = nc.declare_dram_parameter("wmat", [FBLK, NBLK], f32, isOutput=False)
    ones01 = nc.declare_dram_parameter("ones01", [FBLK, 32], f16, isOutput=False)
    # twk4[k, T*128 + m*32 + p'] = tw[((T*4+k)*4 + m)*32 + p']
    twk4 = nc.declare_dram_parameter("twk4", [4, NBANK * 128], f16, isOutput=False)
    # t4 = kron(I4, t): t4[k, gq_l*BS + b] = (k == gq_l) * t[b]
    t4 = nc.declare_dram_parameter("t4", [4, 4 * BS], f16, isOutput=False)
    out_dev = nc.declare_dram_parameter(
        "out_dev", [FBLK, NGQ * BS], f16, isOutput=True
    )

    with TileContext(nc) as tc:
        with (
            tc.tile_pool(name="const", bufs=1) as cpool,
            tc.tile_pool(name="stream", bufs=3) as spool,
            tc.tile_pool(name="osb", bufs=3) as opool,
            tc.tile_pool(name="ps", bufs=3, space="PSUM") as ppool,
        ):
            wmat_sb = cpool.tile([FBLK, NBLK], f32)
            nc.sync.dma_start(out=wmat_sb[:], in_=wmat[:])
            ones01_sb = cpool.tile([FBLK, 32], f16)
            nc.sync.dma_start(out=ones01_sb[:], in_=ones01[:])
            twk4_sb = cpool.tile([4, NBANK * 128], f16)
            nc.sync.dma_start(out=twk4_sb[:], in_=twk4[:])
            t4_sb = cpool.tile([4, 4 * BS], f16)
            nc.sync.dma_start(out=t4_sb[:], in_=t4[:])

            for j in range(NSUPER * reps):
                j = j % NSUPER
                x_tile = spool.tile([FBLK, SUPER], f16, tag="x")
                nc.sync.dma_start(
                    out=x_tile[:], in_=xti[:, j * SUPER : (j + 1) * SUPER]
                )
                # y := x * w per 128-feature block (per-partition scalar),
                # in-place; fp16 single-src tensor_scalar runs in 4x mode
                for blk in range(BLKS_PER_SUPER):
                    g = j * BLKS_PER_SUPER + blk
                    nc.vector.tensor_scalar_mul(
                        x_tile[:, blk * BS : (blk + 1) * BS],
                        x_tile[:, blk * BS : (blk + 1) * BS],
                        wmat_sb[:, g : g + 1],
                    )

                # 2 PSUM bank tiles per chunk; each holds 4 x 128 outputs.
                # Per output sub-range m: one (bias, reduce) N=512 matmul
                # pair spanning the bank's 4 groups -- at most one pending
                # accumulation group per bank at any time.
                x4 = x_tile[:].rearrange(
                    "p (gq four b) -> p four gq b", four=BF, b=BS
                )  # [128, m, gq_l(+tl*4), b]
                for tl in range(BLKS_PER_SUPER // 16):
                    T = j * (BLKS_PER_SUPER // 16) + tl  # global bank-tile index
                    ps = ppool.tile([FBLK, 4, BS], f32, tag="ps")
                    for m in range(4):
                        nc.tensor.matmul(
                            ps[32 * m : 32 * (m + 1), :, :],
                            twk4_sb[:, T * 128 + m * 32 : T * 128 + (m + 1) * 32],
                            t4_sb[:],
                            start=True,
                            stop=False,
                            tile_position=(0, 32 * m),
                        )
                        nc.tensor.matmul(
                            ps[32 * m : 32 * (m + 1), :, :],
                            ones01_sb[:],
                            x4[:, m, tl * 4 : (tl + 1) * 4, :],
                            start=False,
                            stop=True,
                            tile_position=(0, 32 * m),
                        )
                    out_sb = opool.tile([FBLK, 4 * BS], f16, tag="osb")
                    nc.scalar.copy(
                        out=out_sb[:], in_=ps[:].rearrange("p q n -> p (q n)")
                    )
                    nc.gpsimd.dma_start(
                        out=out_dev[:, T * 4 * BS : (T + 1) * 4 * BS],
                        in_=out_sb[:],
                    )
    return nc


def _legalize_waits(nc):
    """Walrus codegen only fits one sync-wait on several instruction
    structs (matmul load-weights, tensor-scalar, nop/drain ...). Move
    excess waits onto same-engine NoOps inserted right before."""
    import concourse.mybir as mybir

    for fn in nc.m.functions:
        for blk in fn.blocks:
            new_insts = []
            for inst in blk.instructions:
                si = inst.sync_info
                if (
                    si is not None
                    and len(si.on_wait) > 1
                    and not isinstance(inst, mybir.InstNoOp)
                ):
                    waits = list(si.on_wait)
                    for k, w in enumerate(waits[:-1]):
                        new_insts.append(
                            mybir.InstNoOp(
                                name=f"{inst.name}-nw{k}",
                                ins=[],
                                outs=[],
                                engine=inst.engine,
                                sync_info=mybir.SyncInfo(
                                    on_wait=[w], on_update=[]
                                ),
                            )
                        )
                    inst.sync_info = mybir.SyncInfo(
                        on_wait=[waits[-1]], on_update=list(si.on_update)
                    )
                new_insts.append(inst)
            blk.instructions = new_insts


def get_nc():
    if "nc" not in _cache:
        nc = _build()
        _legalize_waits(nc)
        _cache["nc"] = nc
    return _cache["nc"]


def make_in_maps(x, t, weight_vals, t_weights):
    x = np.asarray(x, dtype=np.float32)
    t = np.ascontiguousarray(np.asarray(t, dtype=np.float32))
    w = np.asarray(weight_vals, dtype=np.float32)
    tw = np.asarray(t_weights, dtype=np.float32).reshape(NOUT)
    wmat = np.ascontiguousarray(w.reshape(NBLK, FBLK).T)  # fp32
    ones01 = np.zeros((FBLK, 32), dtype=np.float16)
    ones01[np.arange(FBLK), np.arange(FBLK) // BF] = 1.0
    # twk4[k, T, m, p'] = tw[((T*4+k)*4 + m)*32 + p']
    twk4 = np.ascontiguousarray(
        tw.reshape(NBANK, 4, 4, 32)  # [T, k, m, p']
        .transpose(1, 0, 2, 3)  # [k, T, m, p']
        .reshape(4, NBANK * 128)
        .astype(np.float16)
    )
    in_maps = []
    for i in range(NC):
        xs = x[i * BS : (i + 1) * BS]  # [128, 32768]
        # xti[p, g*128 + b] = xs[b, g*128 + p]
        xti = np.ascontiguousarray(
            xs.reshape(BS, NBLK, FBLK)
            .transpose(2, 1, 0)
            .reshape(FBLK, NBLK * BS)
            .astype(np.float16)
        )
        t4 = np.ascontiguousarray(
            np.kron(np.eye(4, dtype=np.float32), t[i * BS : (i + 1) * BS]).astype(
                np.float16
            )
        )
        in_maps.append(
            {"xti": xti, "wmat": wmat, "ones01": ones01, "twk4": twk4, "t4": t4}
        )
    return in_maps


def _unpack_out(out_dev):
    # out_dev [128, 64*128] with dims [pi, (gq, b)]; o = gq*128 + pi
    o = np.asarray(out_dev).astype(np.float32)
    o = o.reshape(FBLK, NGQ, BS).transpose(2, 1, 0)  # [b, gq, pi]
    return np.ascontiguousarray(o.reshape(BS, NOUT))


def kernel(x, t, weight_vals, t_weights):
    from concourse.bass_utils import run_bass_kernel_spmd

    nc = get_nc()
    in_maps = make_in_maps(x, t, weight_vals, t_weights)
    res = run_bass_kernel_spmd(nc, in_maps, list(range(NC)))
    return np.ascontiguousarray(
        np.concatenate([_unpack_out(r["out_dev"]) for r in res.results], axis=0)
    )
